# revision 8
# baseline (speedup 1.0000x reference)
"""Trainium2 kernel for the sobel-perception CNN cell.

Computation (per pixel, circular 3x3 stencil):
    perc = [sobel_x * x, sobel_y * x, x]            # 48 channels
    hidden = relu(W1 @ perc + b1)                   # 128 channels
    out    = W2 @ hidden + b2                       # 16 channels

The depthwise sobel convs share one 2d kernel across channels, so they
commute with the 1x1 channel-mixing conv: folding them into W1 gives
hidden = relu(sum_{dy,dx} M[dy,dx] @ x_shift(dy,dx) + b1).

Device layout (v2):
  * 4-row window per output-row-pair: partitions hold [dr(4) x ch(16)] = 64,
    dy folded into the partition stack, dx a free-dim column offset.
    conv1 = 3 accumulating K=64 matmuls (dx = -1, 0, +1).
  * TWO windows stacked at partition bases 0 / 64 -> PE row-tiling runs the
    two K=64 matmul chains concurrently (2x effective matmul rate).
  * mm2 (M=16) packs 4 chunks into one PSUM bank via 4x column tiling
    (tile_position cols 0/32/64/96) -> 4 chunks copied out per [128,512] op.
  * bf16 operands: FWL weight loads, half the HBM traffic; PSUM stays f32.
  * relu+bias PSUM->SBUF split between Scalar (activation) and Vector
    (tensor_scalar add-bias/max-0) engines.

Sharding: rows of the 1024x1024 grid split across 8 cores (128 rows each);
the host bakes the circular halos into each core's window slab (2x row
replication), so the device kernel needs no collectives.
"""

import sys

sys.path.insert(0, "/opt/trn_rl_repo")

import ml_dtypes
import numpy as np

import concourse.bass as bass
import concourse.mybir as mybir
from concourse.bass_utils import run_bass_kernel_spmd
from concourse.tile import TileContext

H, W, C, HID = 1024, 1024, 16, 128
NCORES = 8
RPC = H // NCORES  # rows per core
SG = RPC // 4  # supergroups per core (4 output rows each)
WP = W + 2  # padded columns
CH = 512  # matmul free-dim chunk (one PSUM bank of fp32)

_SOBEL_X = np.array([[-1.0, 0.0, 1.0], [-2.0, 0.0, 2.0], [-1.0, 0.0, 1.0]], np.float32)
_SOBEL_Y = np.array([[-1.0, -2.0, -1.0], [0.0, 0.0, 0.0], [1.0, 2.0, 1.0]], np.float32)

F32 = mybir.dt.float32
BF16 = mybir.dt.bfloat16
NPBF16 = ml_dtypes.bfloat16


def build_a_mats(W1: np.ndarray) -> np.ndarray:
    """A[dx][o, dy*16+ch] for dx in (-1, 0, +1) -> shape (3, 128, 48)."""
    W1a, W1b, W1c = W1[:, 0:C], W1[:, C : 2 * C], W1[:, 2 * C : 3 * C]
    A = np.zeros((3, HID, 3 * C), np.float32)
    for dxi in range(3):
        for dyi in range(3):
            m = _SOBEL_X[dyi, dxi] * W1a + _SOBEL_Y[dyi, dxi] * W1b
            if dyi == 1 and dxi == 1:
                m = m + W1c
            A[dxi, :, dyi * C : (dyi + 1) * C] = m
    return A


def build_wt(W1: np.ndarray) -> np.ndarray:
    """lhsT slab [128, 6*128]: wt[64t + dr*16+ch, (dxi*2+i)*128 + o] =
    M[dy=dr-1-i, dx=dxi-1][o, ch] (zero outside 0<=dr-i<=2); both 64-row
    halves (t=0,1) hold the same content for the two PE row-tiles."""
    A = build_a_mats(W1)
    wt = np.zeros((128, 6 * HID), np.float32)
    for dxi in range(3):
        for i in range(2):
            col0 = (dxi * 2 + i) * HID
            for dr in range(4):
                dyi = dr - i
                if 0 <= dyi <= 2:
                    blk = A[dxi][:, dyi * C : (dyi + 1) * C]  # (128, 16)
                    for t in range(2):
                        p0 = 64 * t + dr * C
                        wt[p0 : p0 + C, col0 : col0 + HID] = blk.T
    return wt


def _hoist_matmul_waits(nc: bass.Bass) -> None:
    """This walrus build's instruction formats hold at most ONE sync wait,
    but Tile emits 2-3 on some instructions.  Hoist excess waits onto
    inserted same-engine NoOps (one wait each) right before the
    instruction — semantically the same blocking point on the in-order
    engine queue."""
    fixn = 0
    for fn in nc.m.functions:
        for blk in fn.blocks:
            needs_fix = any(
                inst.sync_info is not None and len(inst.sync_info.on_wait) > 1
                for inst in blk.instructions
            )
            if not needs_fix:
                continue
            out = []
            for inst in blk.instructions:
                si = inst.sync_info
                if si is not None and len(si.on_wait) > 1:
                    for w in si.on_wait:
                        nop = mybir.InstNoOp(name=f"I-mmfix-{fixn}")
                        fixn += 1
                        nop.engine = inst.engine
                        nop.sync_info = mybir.SyncInfo(on_wait=[w], on_update=[])
                        out.append(nop)
                    si.on_wait = []
                out.append(inst)
            blk.instructions = out


def build_nc(hoist: bool = True) -> bass.Bass:
    nc = bass.Bass()
    xw = nc.declare_dram_parameter("xw", [128, SG, WP], BF16, isOutput=False)
    wt = nc.declare_dram_parameter("wt", [128, 6 * HID], BF16, isOutput=False)
    w2t = nc.declare_dram_parameter("w2t", [HID, C], BF16, isOutput=False)
    b1 = nc.declare_dram_parameter("b1", [HID, 1], F32, isOutput=False)
    # raw [s][t][strip j = 2i+h][ch][c] layout; host reassembles rows/cols
    out = nc.declare_dram_parameter("out", [SG, 2, 128, CH], F32, isOutput=True)

    with TileContext(nc) as tc:
        with (
            tc.tile_pool(name="const", bufs=1) as cpool,
            tc.tile_pool(name="xrows", bufs=6) as xpool,
            tc.tile_pool(name="hid", bufs=8) as hpool,
            tc.tile_pool(name="stage", bufs=6) as spool,
            tc.tile_pool(name="cps", bufs=2, space="PSUM") as cps,
            tc.tile_pool(name="ops", bufs=4, space="PSUM") as ops,
        ):
            wt_t = cpool.tile([128, 6 * HID], BF16)
            nc.sync.dma_start(out=wt_t[:], in_=wt[:])
            w2t_t = cpool.tile([HID, C], BF16)
            nc.sync.dma_start(out=w2t_t[:], in_=w2t[:])
            b1_t = cpool.tile([HID, 1], F32)
            nc.sync.dma_start(out=b1_t[:], in_=b1[:])

            for s in range(SG):
                win = xpool.tile([128, WP], BF16, tag="xrow", name=f"xw{s}")
                nc.sync.dma_start(out=win[:], in_=xw[:, s, :])

                # conv1: per (t, i): [128, 1024] PSUM tile, 3 accumulating
                # K=64 matmuls per 512-chunk; t=0/1 row-tiles run concurrently
                cv = {}
                for i in range(2):
                    for t in range(2):
                        cv[(t, i)] = cps.tile(
                            [128, 2 * CH], F32, tag="cv", name=f"cv{s}_{t}{i}"
                        )
                for i in range(2):
                    for h in range(2):
                        for dxi in range(3):
                            for t in range(2):
                                nc.tensor.matmul(
                                    cv[(t, i)][:, h * CH : (h + 1) * CH],
                                    wt_t[
                                        64 * t : 64 * t + 64,
                                        (dxi * 2 + i) * HID : (dxi * 2 + i + 1) * HID,
                                    ],
                                    win[64 * t : 64 * t + 64, h * CH + dxi : h * CH + dxi + CH],
                                    start=(dxi == 0),
                                    stop=(dxi == 2),
                                    tile_position=(64 * t, 0),
                                )

                # bias + relu, PSUM -> SBUF bf16; split scalar/vector engines
                hid = {}
                for t in range(2):
                    for i in range(2):
                        ht = hpool.tile([128, 2 * CH], BF16, tag="h", name=f"h{s}_{t}{i}")
                        if i == 0:
                            nc.scalar.activation(
                                ht[:],
                                cv[(t, i)][:],
                                mybir.ActivationFunctionType.Relu,
                                bias=b1_t[:],
                                scale=1.0,
                            )
                        else:
                            nc.vector.tensor_scalar(
                                ht[:],
                                cv[(t, i)][:],
                                b1_t[:],
                                0.0,
                                mybir.AluOpType.add,
                                mybir.AluOpType.max,
                            )
                        hid[(t, i)] = ht

                # mm2: per t, 4 col-tiled chunks into one PSUM bank
                for t in range(2):
                    ot = ops.tile([128, CH], F32, tag="o", name=f"o{s}_{t}")
                    for i in range(2):
                        for h in range(2):
                            j = 2 * i + h
                            nc.tensor.matmul(
                                ot[32 * j : 32 * j + C, :],
                                w2t_t[:],
                                hid[(t, i)][:, h * CH : (h + 1) * CH],
                                start=True,
                                stop=True,
                                tile_position=(0, 32 * j),
                            )
                    st = spool.tile([128, CH], F32, tag="st", name=f"st{s}_{t}")
                    if t == 0:
                        nc.scalar.activation(
                            st[:], ot[:], mybir.ActivationFunctionType.Copy,
                            bias=0.0, scale=1.0,
                        )
                    else:
                        nc.vector.tensor_copy(st[:], ot[:])
                    nc.gpsimd.dma_start(out=out[s, t, :, :], in_=st[:])

    if hoist:
        _hoist_matmul_waits(nc)
    return nc


_NC_CACHE: dict = {}


def _get_nc():
    if "nc" not in _NC_CACHE:
        _NC_CACHE["nc"] = build_nc()
    return _NC_CACHE["nc"]


def host_prepare(state, W1, b1, W2):
    """Build per-core input maps. state: (H, W, C) f32."""
    xt = np.ascontiguousarray(state.transpose(2, 0, 1))  # (C, H, W)
    xtp = np.pad(xt, ((0, 0), (1, 1), (1, 1)), mode="wrap")  # (C, H+2, W+2)
    xtp_bf = xtp.astype(NPBF16)
    wt = build_wt(W1).astype(NPBF16)
    w2t = np.ascontiguousarray(W2.T).astype(NPBF16)  # (128, 16)
    b1c = np.ascontiguousarray(b1.reshape(HID, 1)).astype(np.float32)

    in_maps = []
    for k in range(NCORES):
        r0 = k * RPC
        # xw[64t + dr*16 + ch, s, c] = xtp[ch, r0 + 4s + 2t + dr, c]
        slab = np.empty((128, SG, WP), NPBF16)
        for t in range(2):
            for dr in range(4):
                p0 = 64 * t + dr * C
                rbase = r0 + 2 * t + dr
                slab[p0 : p0 + C] = xtp_bf[:, rbase : rbase + 4 * SG : 4, :]
        in_maps.append(
            {
                "xw": np.ascontiguousarray(slab),
                "wt": wt,
                "w2t": w2t,
                "b1": b1c,
            }
        )
    return in_maps


def assemble_out(results, b2):
    """results[k]["out"]: (SG, 2, 128, CH) raw strips -> (H, W, C) + b2."""
    cores = []
    for k in range(NCORES):
        raw = np.asarray(results[k]["out"])  # (SG, 2, 128, CH)
        v = raw.reshape(SG, 2, 2, 2, 32, CH)[:, :, :, :, :C, :]  # s,t,i,h,ch,c
        core = v.transpose(0, 1, 2, 4, 3, 5).reshape(SG * 4, C, W)  # row,ch,col
        cores.append(core)
    out_t = np.concatenate(cores, axis=0)  # (H, C, W)
    return np.ascontiguousarray(
        out_t.transpose(0, 2, 1) + b2[None, None, :]
    ).astype(np.float32)


def kernel(state, W1, b1, W2, b2, **extra):
    state = np.asarray(state, np.float32)
    W1 = np.asarray(W1, np.float32)
    b1 = np.asarray(b1, np.float32)
    W2 = np.asarray(W2, np.float32)
    b2 = np.asarray(b2, np.float32)

    nc = _get_nc()
    in_maps = host_prepare(state, W1, b1, W2)
    res = run_bass_kernel_spmd(nc, in_maps, core_ids=list(range(NCORES)))
    return assemble_out(res.results, b2)


if __name__ == "__main__":
    rng = np.random.default_rng(0)
    state = rng.standard_normal((H, W, C), dtype=np.float32)
    W1 = rng.standard_normal((HID, 3 * C), dtype=np.float32) * 0.1
    b1v = rng.standard_normal(HID).astype(np.float32) * 0.1
    W2 = rng.standard_normal((C, HID), dtype=np.float32) * 0.1
    b2v = rng.standard_normal(C).astype(np.float32) * 0.1
    out = kernel(state, W1, b1v, W2, b2v)
    print(out.shape, out.dtype)


# revision 9
# speedup vs baseline: 1.8887x; 1.8887x over previous
"""Trainium2 kernel for the sobel-perception CNN cell.

Computation (per pixel, circular 3x3 stencil):
    perc = [sobel_x * x, sobel_y * x, x]            # 48 channels
    hidden = relu(W1 @ perc + b1)                   # 128 channels
    out    = W2 @ hidden + b2                       # 16 channels

The depthwise sobel convs share one 2d kernel across channels, so they
commute with the 1x1 channel-mixing conv: folding them into W1 gives
hidden = relu(sum_{dy,dx} M[dy,dx] @ x_shift(dy,dx) + b1).

Device layout (v4):
  * Window per output-row-pair: partitions [dxv(2) x dr(4) x ch(16)] = 128,
    dy folded into the partition stack; the two dxv blocks hold the row
    data at column shifts 0 / +1.
  * mm_a: one K=128 matmul covers dx = -1 (dxv0) and dx = 0 (dxv1).
  * mm_b: dx = +1, K=64.  Emitted in row-disjoint pairs — chunk A reads
    the dxv0 block at free offset +2 (partitions 0-63), chunk B reads the
    dxv1 block at offset +1 (partitions 64-127) — so consecutive mm_b's
    execute concurrently in the PE array (disjoint row groups).
  * mm2 (M=16) packs 4 chunks into one PSUM bank via 4x column tiling
    (tile_position cols 0/32/64/96) -> all 4 copied out per [128,512] op.
  * bf16 operands (PSUM stays f32); relu+bias PSUM->SBUF split between
    Scalar (activation) and Vector (tensor_scalar) engines.
  * One batched 256KB output DMA per group; host reassembles the strips.

Sharding: rows of the 1024x1024 grid split across 8 cores (128 rows each);
the host bakes the circular halos into each core's window slab, so the
device kernel needs no collectives.
"""

import sys

sys.path.insert(0, "/opt/trn_rl_repo")

import ml_dtypes
import numpy as np

import concourse.bass as bass
import concourse.mybir as mybir
from concourse.bass_utils import run_bass_kernel_spmd
from concourse.tile import TileContext

H, W, C, HID = 1024, 1024, 16, 128
NCORES = 8
RPC = H // NCORES  # rows per core
NG = RPC // 2  # groups per core (2 output rows each)
WP = W + 2  # window free length
CH = 512  # matmul free-dim chunk (one PSUM bank of fp32)

_SOBEL_X = np.array([[-1.0, 0.0, 1.0], [-2.0, 0.0, 2.0], [-1.0, 0.0, 1.0]], np.float32)
_SOBEL_Y = np.array([[-1.0, -2.0, -1.0], [0.0, 0.0, 0.0], [1.0, 2.0, 1.0]], np.float32)

F32 = mybir.dt.float32
BF16 = mybir.dt.bfloat16
NPBF16 = ml_dtypes.bfloat16


def build_a_mats(W1: np.ndarray) -> np.ndarray:
    """A[dx][o, dy*16+ch] for dx in (-1, 0, +1) -> shape (3, 128, 48)."""
    W1a, W1b, W1c = W1[:, 0:C], W1[:, C : 2 * C], W1[:, 2 * C : 3 * C]
    A = np.zeros((3, HID, 3 * C), np.float32)
    for dxi in range(3):
        for dyi in range(3):
            m = _SOBEL_X[dyi, dxi] * W1a + _SOBEL_Y[dyi, dxi] * W1b
            if dyi == 1 and dxi == 1:
                m = m + W1c
            A[dxi, :, dyi * C : (dyi + 1) * C] = m
    return A


def build_window_mats(W1: np.ndarray):
    """Fold matrices for the 4-row x 2-dx window layout.

    Window partition p = dxv*64 + dr*16 + ch holds xtp[ch, rbase+dr, c+dxv].
    Output row i (i in {0,1} within the pair) uses dy = dr-1-i; mm_a covers
    dx = dxv-1 in one K=128 pass; mm_b adds dx=+1 (K=64).  Returns
    (aa, ab): aa[i] is [128, 128] lhsT for mm_a, ab[i] is [64, 128]."""
    A = build_a_mats(W1)
    aa = np.zeros((2, 128, HID), np.float32)
    ab = np.zeros((2, 64, HID), np.float32)
    for i in range(2):
        for dxv in range(2):
            for dr in range(4):
                dyi = dr - i
                if 0 <= dyi <= 2:
                    blk = A[dxv][:, dyi * C : (dyi + 1) * C]  # (128, 16)
                    aa[i, dxv * 64 + dr * C : dxv * 64 + dr * C + C] = blk.T
        for dr in range(4):
            dyi = dr - i
            if 0 <= dyi <= 2:
                blk = A[2][:, dyi * C : (dyi + 1) * C]
                ab[i, dr * C : dr * C + C] = blk.T
    return aa, ab


def _hoist_matmul_waits(nc: bass.Bass) -> None:
    """This walrus build's instruction formats hold at most ONE sync wait,
    but Tile emits 2-3 on some instructions.  Hoist excess waits onto
    inserted same-engine NoOps (one wait each) right before the
    instruction — semantically the same blocking point on the in-order
    engine queue."""
    fixn = 0
    for fn in nc.m.functions:
        for blk in fn.blocks:
            needs_fix = any(
                inst.sync_info is not None and len(inst.sync_info.on_wait) > 1
                for inst in blk.instructions
            )
            if not needs_fix:
                continue
            out = []
            for inst in blk.instructions:
                si = inst.sync_info
                if si is not None and len(si.on_wait) > 1:
                    for w in si.on_wait:
                        nop = mybir.InstNoOp(name=f"I-mmfix-{fixn}")
                        fixn += 1
                        nop.engine = inst.engine
                        nop.sync_info = mybir.SyncInfo(on_wait=[w], on_update=[])
                        out.append(nop)
                    si.on_wait = []
                out.append(inst)
            blk.instructions = out


def build_nc(hoist: bool = True) -> bass.Bass:
    nc = bass.Bass()
    xw = nc.declare_dram_parameter("xw", [128, NG, WP], BF16, isOutput=False)
    ata = nc.declare_dram_parameter("ata", [128, 2 * HID], BF16, isOutput=False)
    atb = nc.declare_dram_parameter("atb", [128, 2 * HID], BF16, isOutput=False)
    w2t = nc.declare_dram_parameter("w2t", [HID, C], BF16, isOutput=False)
    b1 = nc.declare_dram_parameter("b1", [HID, 1], F32, isOutput=False)
    # raw [g][strip j = 2i+h][ch][c] layout; host reassembles rows/cols
    out = nc.declare_dram_parameter("out", [NG, 128, CH], F32, isOutput=True)

    with TileContext(nc) as tc:
        with (
            tc.tile_pool(name="const", bufs=1) as cpool,
            tc.tile_pool(name="xrows", bufs=6) as xpool,
            tc.tile_pool(name="hid", bufs=6) as hpool,
            tc.tile_pool(name="stage", bufs=4) as spool,
            tc.tile_pool(name="cps", bufs=3, space="PSUM") as cps,
            tc.tile_pool(name="ops", bufs=2, space="PSUM") as ops,
        ):
            ata_t = cpool.tile([128, 2 * HID], BF16)
            nc.sync.dma_start(out=ata_t[:], in_=ata[:])
            atb_t = cpool.tile([128, 2 * HID], BF16)
            nc.sync.dma_start(out=atb_t[:], in_=atb[:])
            w2t_t = cpool.tile([HID, C], BF16)
            nc.sync.dma_start(out=w2t_t[:], in_=w2t[:])
            b1_t = cpool.tile([HID, 1], F32)
            nc.sync.dma_start(out=b1_t[:], in_=b1[:])

            for g in range(NG):
                win = xpool.tile([128, WP], BF16, tag="xrow", name=f"xw{g}")
                nc.sync.dma_start(out=win[:], in_=xw[:, g, :])

                # conv1: per i: [128, 1024] PSUM tile; chunks (i, h).
                # mm_a K=128 (start), mm_b K=64 (stop).  mm_b emitted in
                # row-disjoint (i0: rows 0-63, i1: rows 64-127) pairs.
                cv = [
                    cps.tile([128, 2 * CH], F32, tag="cv", name=f"cv{g}_{i}")
                    for i in range(2)
                ]
                for h in range(2):
                    for i in range(2):
                        nc.tensor.matmul(
                            cv[i][:, h * CH : (h + 1) * CH],
                            ata_t[:, i * HID : (i + 1) * HID],
                            win[:, h * CH : h * CH + CH],
                            start=True,
                            stop=False,
                        )
                    # dx=+1 pair: chunk (0,h) via dxv0 block at offset +2,
                    # chunk (1,h) via dxv1 block at offset +1
                    nc.tensor.matmul(
                        cv[0][:, h * CH : (h + 1) * CH],
                        atb_t[0:64, 0:HID],
                        win[0:64, h * CH + 2 : h * CH + 2 + CH],
                        start=False,
                        stop=True,
                        tile_position=(0, 0),
                    )
                    nc.tensor.matmul(
                        cv[1][:, h * CH : (h + 1) * CH],
                        atb_t[64:128, HID : 2 * HID],
                        win[64:128, h * CH + 1 : h * CH + 1 + CH],
                        start=False,
                        stop=True,
                        tile_position=(64, 0),
                    )

                # bias + relu, PSUM -> SBUF bf16; split scalar/vector engines
                hid = []
                for i in range(2):
                    ht = hpool.tile([128, 2 * CH], BF16, tag="h", name=f"h{g}_{i}")
                    if i == 0:
                        nc.scalar.activation(
                            ht[:],
                            cv[i][:],
                            mybir.ActivationFunctionType.Relu,
                            bias=b1_t[:],
                            scale=1.0,
                        )
                    else:
                        nc.vector.tensor_scalar(
                            ht[:],
                            cv[i][:],
                            b1_t[:],
                            0.0,
                            mybir.AluOpType.add,
                            mybir.AluOpType.max,
                        )
                    hid.append(ht)

                # mm2: 4 col-tiled chunks into one PSUM bank
                ot = ops.tile([128, CH], F32, tag="o", name=f"o{g}")
                for i in range(2):
                    for h in range(2):
                        j = 2 * i + h
                        nc.tensor.matmul(
                            ot[32 * j : 32 * j + C, :],
                            w2t_t[:],
                            hid[i][:, h * CH : (h + 1) * CH],
                            start=True,
                            stop=True,
                            tile_position=(0, 32 * j),
                        )
                st = spool.tile([128, CH], F32, tag="st", name=f"st{g}")
                if g % 2 == 0:
                    nc.scalar.activation(
                        st[:], ot[:], mybir.ActivationFunctionType.Copy,
                        bias=0.0, scale=1.0,
                    )
                else:
                    nc.vector.tensor_copy(st[:], ot[:])
                nc.gpsimd.dma_start(out=out[g, :, :], in_=st[:])

    if hoist:
        _hoist_matmul_waits(nc)
    return nc


_NC_CACHE: dict = {}


def _get_nc():
    if "nc" not in _NC_CACHE:
        _NC_CACHE["nc"] = build_nc()
    return _NC_CACHE["nc"]


def host_prepare(state, W1, b1, W2):
    """Build per-core input maps. state: (H, W, C) f32."""
    xt = np.ascontiguousarray(state.transpose(2, 0, 1))  # (C, H, W)
    xtp = np.pad(xt, ((0, 0), (1, 1), (1, 2)), mode="wrap")  # (C, H+2, W+3)
    xtp_bf = xtp.astype(NPBF16)
    aa, ab = build_window_mats(W1)
    ata = np.ascontiguousarray(
        np.concatenate([aa[0], aa[1]], axis=1)
    ).astype(NPBF16)  # (128, 256)
    # ab duplicated into both 64-row halves for the row-disjoint mm_b pairs
    atbf = np.zeros((128, 2 * HID), np.float32)
    atbf[0:64] = np.concatenate([ab[0], ab[1]], axis=1)
    atbf[64:128] = atbf[0:64]
    atb = atbf.astype(NPBF16)
    w2t = np.ascontiguousarray(W2.T).astype(NPBF16)  # (128, 16)
    b1c = np.ascontiguousarray(b1.reshape(HID, 1)).astype(np.float32)

    in_maps = []
    for k in range(NCORES):
        r0 = k * RPC
        # xw[dxv*64 + dr*16 + ch, g, q] = xtp[ch, r0 + 2g + dr, q + dxv]
        slab = np.empty((128, NG, WP), NPBF16)
        for dxv in range(2):
            for dr in range(4):
                p0 = dxv * 64 + dr * C
                slab[p0 : p0 + C] = xtp_bf[
                    :, r0 + dr : r0 + dr + 2 * NG : 2, dxv : dxv + WP
                ]
        in_maps.append(
            {
                "xw": np.ascontiguousarray(slab),
                "ata": ata,
                "atb": atb,
                "w2t": w2t,
                "b1": b1c,
            }
        )
    return in_maps


def assemble_out(results, b2):
    """results[k]["out"]: (NG, 128, CH) raw strips -> (H, W, C) + b2."""
    cores = []
    for k in range(NCORES):
        raw = np.asarray(results[k]["out"])  # (NG, 128, CH)
        v = raw.reshape(NG, 2, 2, 32, CH)[:, :, :, :C, :]  # g,i,h,ch,c
        core = v.transpose(0, 1, 3, 2, 4).reshape(RPC, C, W)  # row,ch,col
        cores.append(core)
    out_t = np.concatenate(cores, axis=0)  # (H, C, W)
    return np.ascontiguousarray(
        out_t.transpose(0, 2, 1) + b2[None, None, :]
    ).astype(np.float32)


def kernel(state, W1, b1, W2, b2, **extra):
    state = np.asarray(state, np.float32)
    W1 = np.asarray(W1, np.float32)
    b1 = np.asarray(b1, np.float32)
    W2 = np.asarray(W2, np.float32)
    b2 = np.asarray(b2, np.float32)

    nc = _get_nc()
    in_maps = host_prepare(state, W1, b1, W2)
    res = run_bass_kernel_spmd(nc, in_maps, core_ids=list(range(NCORES)))
    return assemble_out(res.results, b2)


if __name__ == "__main__":
    rng = np.random.default_rng(0)
    state = rng.standard_normal((H, W, C), dtype=np.float32)
    W1 = rng.standard_normal((HID, 3 * C), dtype=np.float32) * 0.1
    b1v = rng.standard_normal(HID).astype(np.float32) * 0.1
    W2 = rng.standard_normal((C, HID), dtype=np.float32) * 0.1
    b2v = rng.standard_normal(C).astype(np.float32) * 0.1
    out = kernel(state, W1, b1v, W2, b2v)
    print(out.shape, out.dtype)


# revision 14
# speedup vs baseline: 2.0476x; 1.0842x over previous
"""Trainium2 kernel for the sobel-perception CNN cell.

Computation (per pixel, circular 3x3 stencil):
    perc = [sobel_x * x, sobel_y * x, x]            # 48 channels
    hidden = relu(W1 @ perc + b1)                   # 128 channels
    out    = W2 @ hidden + b2                       # 16 channels

The depthwise sobel convs share one 2d kernel across channels, so they
commute with the 1x1 channel-mixing conv: folding them into W1 gives
hidden = relu(sum_{dy,dx} M[dy,dx] @ x_shift(dy,dx) + b1).

Device layout (v4):
  * Window per output-row-pair: partitions [dxv(2) x dr(4) x ch(16)] = 128,
    dy folded into the partition stack; the two dxv blocks hold the row
    data at column shifts 0 / +1.
  * mm_a: one K=128 matmul covers dx = -1 (dxv0) and dx = 0 (dxv1).
  * mm_b: dx = +1, K=64.  Emitted in row-disjoint pairs — chunk A reads
    the dxv0 block at free offset +2 (partitions 0-63), chunk B reads the
    dxv1 block at offset +1 (partitions 64-127) — so consecutive mm_b's
    execute concurrently in the PE array (disjoint row groups).
  * mm2 (M=16) packs 4 chunks into one PSUM bank via 4x column tiling
    (tile_position cols 0/32/64/96) -> all 4 copied out per [128,512] op.
  * bf16 operands (PSUM stays f32); relu+bias PSUM->SBUF split between
    Scalar (activation) and Vector (tensor_scalar) engines.
  * One batched 256KB output DMA per group; host reassembles the strips.

Sharding: rows of the 1024x1024 grid split across 8 cores (128 rows each);
the host bakes the circular halos into each core's window slab, so the
device kernel needs no collectives.
"""

import sys

sys.path.insert(0, "/opt/trn_rl_repo")

import ml_dtypes
import numpy as np

import concourse.bass as bass
import concourse.mybir as mybir
from concourse.bass_utils import run_bass_kernel_spmd
from concourse.tile import TileContext

H, W, C, HID = 1024, 1024, 16, 128
NCORES = 8
RPC = H // NCORES  # rows per core
NG = RPC // 2  # groups per core (2 output rows each)
WP = W + 2  # window free length
CH = 512  # matmul free-dim chunk (one PSUM bank of fp32)

_SOBEL_X = np.array([[-1.0, 0.0, 1.0], [-2.0, 0.0, 2.0], [-1.0, 0.0, 1.0]], np.float32)
_SOBEL_Y = np.array([[-1.0, -2.0, -1.0], [0.0, 0.0, 0.0], [1.0, 2.0, 1.0]], np.float32)

F32 = mybir.dt.float32
BF16 = mybir.dt.bfloat16
NPBF16 = ml_dtypes.bfloat16


def build_a_mats(W1: np.ndarray) -> np.ndarray:
    """A[dx][o, dy*16+ch] for dx in (-1, 0, +1) -> shape (3, 128, 48)."""
    W1a, W1b, W1c = W1[:, 0:C], W1[:, C : 2 * C], W1[:, 2 * C : 3 * C]
    A = np.zeros((3, HID, 3 * C), np.float32)
    for dxi in range(3):
        for dyi in range(3):
            m = _SOBEL_X[dyi, dxi] * W1a + _SOBEL_Y[dyi, dxi] * W1b
            if dyi == 1 and dxi == 1:
                m = m + W1c
            A[dxi, :, dyi * C : (dyi + 1) * C] = m
    return A


def build_window_mats(W1: np.ndarray):
    """Fold matrices for the 4-row x 2-dx window layout.

    Window partition p = dxv*64 + dr*16 + ch holds xtp[ch, rbase+dr, c+dxv].
    Output row i (i in {0,1} within the pair) uses dy = dr-1-i; mm_a covers
    dx = dxv-1 in one K=128 pass; mm_b adds dx=+1 (K=64).  Returns
    (aa, ab): aa[i] is [128, 128] lhsT for mm_a, ab[i] is [64, 128]."""
    A = build_a_mats(W1)
    aa = np.zeros((2, 128, HID), np.float32)
    ab = np.zeros((2, 64, HID), np.float32)
    for i in range(2):
        for dxv in range(2):
            for dr in range(4):
                dyi = dr - i
                if 0 <= dyi <= 2:
                    blk = A[dxv][:, dyi * C : (dyi + 1) * C]  # (128, 16)
                    aa[i, dxv * 64 + dr * C : dxv * 64 + dr * C + C] = blk.T
        for dr in range(4):
            dyi = dr - i
            if 0 <= dyi <= 2:
                blk = A[2][:, dyi * C : (dyi + 1) * C]
                ab[i, dr * C : dr * C + C] = blk.T
    return aa, ab


def _hoist_matmul_waits(nc: bass.Bass) -> None:
    """This walrus build's instruction formats hold at most ONE sync wait,
    but Tile emits 2-3 on some instructions.  Hoist excess waits onto
    inserted same-engine NoOps (one wait each) right before the
    instruction — semantically the same blocking point on the in-order
    engine queue."""
    fixn = 0
    for fn in nc.m.functions:
        for blk in fn.blocks:
            needs_fix = any(
                inst.sync_info is not None and len(inst.sync_info.on_wait) > 1
                for inst in blk.instructions
            )
            if not needs_fix:
                continue
            out = []
            for inst in blk.instructions:
                si = inst.sync_info
                if si is not None and len(si.on_wait) > 1:
                    for w in si.on_wait:
                        nop = mybir.InstNoOp(name=f"I-mmfix-{fixn}")
                        fixn += 1
                        nop.engine = inst.engine
                        nop.sync_info = mybir.SyncInfo(on_wait=[w], on_update=[])
                        out.append(nop)
                    si.on_wait = []
                out.append(inst)
            blk.instructions = out


def build_nc(hoist: bool = True) -> bass.Bass:
    nc = bass.Bass()
    xw = nc.declare_dram_parameter("xw", [128, NG, WP], BF16, isOutput=False)
    ata = nc.declare_dram_parameter("ata", [128, 2 * HID], BF16, isOutput=False)
    atb = nc.declare_dram_parameter("atb", [128, 2 * HID], BF16, isOutput=False)
    w2t = nc.declare_dram_parameter("w2t", [HID, C], BF16, isOutput=False)
    b1 = nc.declare_dram_parameter("b1", [HID, 1], F32, isOutput=False)
    # raw [g][strip j = 2i+h][ch][c] layout; host reassembles rows/cols
    out = nc.declare_dram_parameter("out", [NG, 128, CH], BF16, isOutput=True)

    with TileContext(nc) as tc:
        with (
            tc.tile_pool(name="const", bufs=1) as cpool,
            tc.tile_pool(name="xrows", bufs=6) as xpool,
            tc.tile_pool(name="hid", bufs=6) as hpool,
            tc.tile_pool(name="stage", bufs=4) as spool,
            tc.tile_pool(name="cps", bufs=3, space="PSUM") as cps,
            tc.tile_pool(name="ops", bufs=2, space="PSUM") as ops,
        ):
            ata_t = cpool.tile([128, 2 * HID], BF16)
            nc.sync.dma_start(out=ata_t[:], in_=ata[:])
            atb_t = cpool.tile([128, 2 * HID], BF16)
            nc.sync.dma_start(out=atb_t[:], in_=atb[:])
            w2t_t = cpool.tile([HID, C], BF16)
            nc.sync.dma_start(out=w2t_t[:], in_=w2t[:])
            b1_t = cpool.tile([HID, 1], F32)
            nc.sync.dma_start(out=b1_t[:], in_=b1[:])

            for g in range(NG):
                win = xpool.tile([128, WP], BF16, tag="xrow", name=f"xw{g}")
                nc.sync.dma_start(out=win[:], in_=xw[:, g, :])

                # conv1: per i: [128, 1024] PSUM tile; chunks (i, h).
                # mm_a K=128 (start), mm_b K=64 (stop).  mm_b emitted in
                # row-disjoint (i0: rows 0-63, i1: rows 64-127) pairs.
                cv = [
                    cps.tile([128, 2 * CH], F32, tag="cv", name=f"cv{g}_{i}")
                    for i in range(2)
                ]
                # mm_a i-outer: consecutive matmuls share one lhsT
                for i in range(2):
                    for h in range(2):
                        nc.tensor.matmul(
                            cv[i][:, h * CH : (h + 1) * CH],
                            ata_t[:, i * HID : (i + 1) * HID],
                            win[:, h * CH : h * CH + CH],
                            start=True,
                            stop=False,
                        )
                # dx=+1 pairs: chunk (0,h) via dxv0 block at offset +2,
                # chunk (1,h) via dxv1 block at offset +1 (row-disjoint)
                for h in range(2):
                    nc.tensor.matmul(
                        cv[0][:, h * CH : (h + 1) * CH],
                        atb_t[0:64, 0:HID],
                        win[0:64, h * CH + 2 : h * CH + 2 + CH],
                        start=False,
                        stop=True,
                        tile_position=(0, 0),
                    )
                    nc.tensor.matmul(
                        cv[1][:, h * CH : (h + 1) * CH],
                        atb_t[64:128, HID : 2 * HID],
                        win[64:128, h * CH + 1 : h * CH + 1 + CH],
                        start=False,
                        stop=True,
                        tile_position=(64, 0),
                    )

                # bias + relu, PSUM -> SBUF bf16; split scalar/vector engines
                # ACT is faster per relu than DVE; give it 3 of every 4
                # (i1 relu alternates), DVE gets the other + all copies
                hid = []
                for i in range(2):
                    ht = hpool.tile([128, 2 * CH], BF16, tag="h", name=f"h{g}_{i}")
                    if i == 0 or g % 2 == 1:
                        nc.scalar.activation(
                            ht[:],
                            cv[i][:],
                            mybir.ActivationFunctionType.Relu,
                            bias=b1_t[:],
                            scale=1.0,
                        )
                    else:
                        nc.vector.tensor_scalar(
                            ht[:],
                            cv[i][:],
                            b1_t[:],
                            0.0,
                            mybir.AluOpType.add,
                            mybir.AluOpType.max,
                        )
                    hid.append(ht)

                # mm2: 4 col-tiled chunks into one PSUM bank
                ot = ops.tile([128, CH], F32, tag="o", name=f"o{g}")
                for i in range(2):
                    for h in range(2):
                        j = 2 * i + h
                        nc.tensor.matmul(
                            ot[32 * j : 32 * j + C, :],
                            w2t_t[:],
                            hid[i][:, h * CH : (h + 1) * CH],
                            start=True,
                            stop=True,
                            tile_position=(0, 32 * j),
                        )
                st = spool.tile([128, CH], BF16, tag="st", name=f"st{g}")
                nc.vector.tensor_copy(st[:], ot[:])
                nc.gpsimd.dma_start(out=out[g, :, :], in_=st[:])

    if hoist:
        _hoist_matmul_waits(nc)
    return nc


_NC_CACHE: dict = {}


def _get_nc():
    if "nc" not in _NC_CACHE:
        _NC_CACHE["nc"] = build_nc()
    return _NC_CACHE["nc"]


def host_prepare(state, W1, b1, W2):
    """Build per-core input maps. state: (H, W, C) f32."""
    xt = np.ascontiguousarray(state.transpose(2, 0, 1))  # (C, H, W)
    xtp = np.pad(xt, ((0, 0), (1, 1), (1, 2)), mode="wrap")  # (C, H+2, W+3)
    xtp_bf = xtp.astype(NPBF16)
    aa, ab = build_window_mats(W1)
    ata = np.ascontiguousarray(
        np.concatenate([aa[0], aa[1]], axis=1)
    ).astype(NPBF16)  # (128, 256)
    # ab duplicated into both 64-row halves for the row-disjoint mm_b pairs
    atbf = np.zeros((128, 2 * HID), np.float32)
    atbf[0:64] = np.concatenate([ab[0], ab[1]], axis=1)
    atbf[64:128] = atbf[0:64]
    atb = atbf.astype(NPBF16)
    w2t = np.ascontiguousarray(W2.T).astype(NPBF16)  # (128, 16)
    b1c = np.ascontiguousarray(b1.reshape(HID, 1)).astype(np.float32)

    in_maps = []
    for k in range(NCORES):
        r0 = k * RPC
        # xw[dxv*64 + dr*16 + ch, g, q] = xtp[ch, r0 + 2g + dr, q + dxv]
        slab = np.empty((128, NG, WP), NPBF16)
        for dxv in range(2):
            for dr in range(4):
                p0 = dxv * 64 + dr * C
                slab[p0 : p0 + C] = xtp_bf[
                    :, r0 + dr : r0 + dr + 2 * NG : 2, dxv : dxv + WP
                ]
        in_maps.append(
            {
                "xw": np.ascontiguousarray(slab),
                "ata": ata,
                "atb": atb,
                "w2t": w2t,
                "b1": b1c,
            }
        )
    return in_maps


def assemble_out(results, b2):
    """results[k]["out"]: (NG, 128, CH) raw strips -> (H, W, C) + b2."""
    cores = []
    for k in range(NCORES):
        raw = np.asarray(results[k]["out"]).astype(np.float32)  # (NG, 128, CH)
        v = raw.reshape(NG, 2, 2, 32, CH)[:, :, :, :C, :]  # g,i,h,ch,c
        core = v.transpose(0, 1, 3, 2, 4).reshape(RPC, C, W)  # row,ch,col
        cores.append(core)
    out_t = np.concatenate(cores, axis=0)  # (H, C, W)
    return np.ascontiguousarray(
        out_t.transpose(0, 2, 1) + b2[None, None, :]
    ).astype(np.float32)


def kernel(state, W1, b1, W2, b2, **extra):
    state = np.asarray(state, np.float32)
    W1 = np.asarray(W1, np.float32)
    b1 = np.asarray(b1, np.float32)
    W2 = np.asarray(W2, np.float32)
    b2 = np.asarray(b2, np.float32)

    nc = _get_nc()
    in_maps = host_prepare(state, W1, b1, W2)
    res = run_bass_kernel_spmd(nc, in_maps, core_ids=list(range(NCORES)))
    return assemble_out(res.results, b2)


if __name__ == "__main__":
    rng = np.random.default_rng(0)
    state = rng.standard_normal((H, W, C), dtype=np.float32)
    W1 = rng.standard_normal((HID, 3 * C), dtype=np.float32) * 0.1
    b1v = rng.standard_normal(HID).astype(np.float32) * 0.1
    W2 = rng.standard_normal((C, HID), dtype=np.float32) * 0.1
    b2v = rng.standard_normal(C).astype(np.float32) * 0.1
    out = kernel(state, W1, b1v, W2, b2v)
    print(out.shape, out.dtype)
